# revision 1
# baseline (speedup 1.0000x reference)
"""3x3 VALID conv (NCHW) on 8 Trainium2 NeuronCores, data-parallel on batch.

Contract: kernel(img, filtro) takes the FULL inputs
  img    [32, 128, 56, 56] f32
  filtro [256, 128, 3, 3]  f32
and returns the FULL output [32, 256, 54, 54] f32.

Strategy (per core, batch shard of 4 images):
- Inputs are host-cast to bf16 (rel-err 2.2e-3 measured, gate 2e-2).
  bf16 matmuls stream 1 col/cycle like fp32r, but their weight loads are
  emitted as standalone Ldweights instructions that the PE's 64-deep
  reorder window hides behind in-flight matmuls - fp32r self-loading
  matmuls instead pay a serial ~107ns reload inside every matmul.
  A post-Tile IR pass (_dedup_ldweights) drops Ldweights that reload the
  stationary already in the array (taps-outer reuses each load 4x).
- img in SBUF channels-on-partitions: [ci=128, n, h, w] bf16 (25KB/part).
- w host-packed to [ci, cb, tap, co128] so each (cb, tap) slice
  [128, 128] is one stationary load.
- Schedule: cb-major; the 24 (image, row-group) tiles per cb are
  processed in waves of 4 (9 rows x 54 = 486 cols = one PSUM bank each),
  taps outer. With 8 PSUM banks, consecutive waves always land on fresh
  banks, so a wave's drains have a full wave (~9us) to finish - no PSUM
  WAR stall and no head-of-line wait on the wave's first Ldweights.
- Drain: DVE copies each bank to an SBUF tile; out DMA goes on the ACT
  HWDGE ring (nc.scalar.dma_start) so stores never head-of-line block
  the SP ring that prefetches the next iteration's images.
- Measured (For_i slope, 8 cores): ~117us/rep steady state vs a
  ~105-115us pure-matmul floor (PE sustains ~2.0GHz under load, P0);
  the fp32r baseline was 147us.
"""
from contextlib import ExitStack

import numpy as np

BATCH, C_IN, C_OUT, H, K = 32, 128, 256, 56, 3
OH = H - K + 1  # 54
N_CORES = 8
PER = BATCH // N_CORES  # 4
RG = 9          # output rows per matmul group; 9*54=486 <= 512-f32 PSUM bank
NG = OH // RG   # 6

_CACHE = {}
DEDUP = True


def _dedup_ldweights(nc):
    """Remove Ldweights whose stationary AP matches the weights already in
    the PE array (tile legalization emits one per matmul; taps-outer reuses
    each load 6x). Any waits/updates on a removed load move to the next PE
    instruction."""
    removed = 0
    for blk in nc.m.functions[0].blocks:
        insts = list(blk.instructions)
        last_w = None
        drop, pending = [], []
        for i, inst in enumerate(insts):
            if str(getattr(inst, "engine", "")) != "EngineType.PE":
                continue
            if pending and inst.opcode in ("Matmult", "Ldweights"):
                si = inst.sync_info
                waits = list(si.on_wait) if si else []
                ups = list(si.on_update) if si else []
                from concourse import mybir as _mb
                for psi in pending:
                    waits += list(psi.on_wait)
                    ups += list(psi.on_update)
                inst.sync_info = _mb.SyncInfo(on_wait=waits, on_update=ups)
                pending = []
            if inst.opcode == "Ldweights":
                sig = str(inst.ins[0])
                if sig == last_w:
                    si = inst.sync_info
                    if si and (len(si.on_wait) or len(si.on_update)):
                        pending.append(si)
                    drop.append(i)
                else:
                    last_w = sig
            elif inst.opcode == "Matmult":
                if inst.ldweights is not False:
                    last_w = str(inst.ins[1])
            else:
                last_w = None
        assert not pending
        for i in reversed(drop):
            del blk.instructions[i]
        removed += len(drop)
    return removed


def _build(reps=1, drop_out=False, drop_mm=False, flat_rhs=False,
           single_w=False, drain_engine="vector", out_ring="scalar",
           wave_sz=4, in_bufs=2, mm_split=1, hoist_in=False):
    import concourse.tile as tile
    from concourse import bacc, mybir

    BF = mybir.dt.bfloat16
    F32 = mybir.dt.float32
    first_chunks = 3

    nc = bacc.Bacc(None, target_bir_lowering=False)
    img = nc.declare_dram_parameter("img", [PER, C_IN, H, H], BF,
                                    isOutput=False)
    w = nc.declare_dram_parameter("w", [C_IN, 2 * K * K * 128], BF,
                                  isOutput=False)
    out = nc.declare_dram_parameter("out", [PER, C_OUT, OH, OH], F32,
                                    isOutput=True)

    with tile.TileContext(nc) as tc:
        with ExitStack() as ctx:
            wpool = ctx.enter_context(
                tc.tile_pool(name="wpool", bufs=in_bufs))
            imgpool = ctx.enter_context(
                tc.tile_pool(name="imgpool", bufs=in_bufs))
            psum_pool = ctx.enter_context(
                tc.tile_pool(name="psum", bufs=8, space="PSUM"))
            outp = ctx.enter_context(tc.tile_pool(
                name="outp", bufs=4 if drain_engine == "bigdma" else 8))

            def load():
                w_sb = wpool.tile([C_IN, 2 * K * K * 128], BF)
                # t0/cb0 block (32KB) first: the first Ldweights needs only
                # this; subtile dep tracking gives it the early sem. Then
                # img0's first row-chunk, then the rest of the cb0 half.
                half = K * K * 128
                nc.sync.dma_start(out=w_sb[:, :128], in_=w[:, :128])
                img_sb = imgpool.tile([C_IN, PER, H, H], BF)
                src = img.rearrange("n c h w -> c n h w")
                bounds = [0]
                step = (H + first_chunks - 1) // first_chunks
                while bounds[-1] < H:
                    bounds.append(min(bounds[-1] + step, H))
                for i, (r0, r1) in enumerate(zip(bounds[:-1], bounds[1:])):
                    nc.sync.dma_start(out=img_sb[:, 0, r0:r1],
                                      in_=src[:, 0, r0:r1])
                    if i == 0:
                        nc.sync.dma_start(out=w_sb[:, 128:half],
                                          in_=w[:, 128:half])
                for n in range(1, PER):
                    nc.sync.dma_start(out=img_sb[:, n], in_=src[:, n])
                nc.sync.dma_start(out=w_sb[:, half:], in_=w[:, half:])
                return w_sb, img_sb

            def body(preloaded=None):
                w_sb, img_sb = preloaded if preloaded else load()

                groups = [(n, g) for n in range(PER) for g in range(NG)]
                for cb in range(2):
                    for w0 in range(0, len(groups), wave_sz):
                        wave = groups[w0:w0 + wave_sz]
                        pss = {ng: psum_pool.tile([128, RG * OH], F32,
                                                  name=f"ps{ng}", tag="ps")
                               for ng in wave}
                        if not drop_mm:
                            for t in range(K * K):
                                ki, kj = divmod(t, K)
                                col = (cb * K * K + t) * 128
                                if single_w:
                                    col = 0
                                lhsT = w_sb[:, col: col + 128]
                                for (n, g) in wave:
                                    if flat_rhs:
                                        rhs = img_sb[
                                            :, n].rearrange(
                                            "p h w -> p (h w)")[:, :RG * OH]
                                    else:
                                        rhs = img_sb[
                                            :, n,
                                            g * RG + ki: g * RG + ki + RG,
                                            kj: kj + OH]
                                    if mm_split == 1:
                                        nc.tensor.matmul(
                                            pss[(n, g)], lhsT, rhs,
                                            start=(t == 0),
                                            stop=(t == K * K - 1))
                                    else:
                                        assert flat_rhs
                                        cw = RG * OH // mm_split
                                        for s in range(mm_split):
                                            nc.tensor.matmul(
                                                pss[(n, g)][
                                                    :, s * cw:(s + 1) * cw],
                                                lhsT,
                                                rhs[:, s * cw:(s + 1) * cw],
                                                start=(t == 0),
                                                stop=(t == K * K - 1))
                        if drop_out or drop_mm:
                            continue
                        dma_eng = (nc.scalar if out_ring == "scalar"
                                   else nc.sync)
                        if drain_engine == "bigdma":
                            # one ob tile + one DMA per same-image run
                            runs = []
                            for (n, g) in wave:
                                if runs and runs[-1][0] == n:
                                    runs[-1][1].append(g)
                                else:
                                    runs.append((n, [g]))
                            for n, gs in runs:
                                ob = outp.tile([128, len(gs) * RG * OH],
                                               F32, name="ob", tag="ob")
                                for i, g in enumerate(gs):
                                    nc.vector.tensor_copy(
                                        ob[:, i * RG * OH:(i + 1) * RG * OH],
                                        pss[(n, g)])
                                nc.scalar.dma_start(
                                    out=out[n, cb * 128:(cb + 1) * 128,
                                            gs[0] * RG:
                                            (gs[0] + len(gs)) * RG],
                                    in_=ob.rearrange(
                                        "p (r x) -> p r x",
                                        r=len(gs) * RG))
                            continue
                        for j, (n, g) in enumerate(wave):
                            dst = out[n, cb * 128:(cb + 1) * 128,
                                      g * RG:(g + 1) * RG]
                            ob = outp.tile([128, RG * OH], F32,
                                           name="ob", tag="ob")
                            eng = drain_engine
                            if eng == "alt":
                                eng = "vector" if j % 2 == 0 else "scalar"
                            if eng == "vector":
                                nc.vector.tensor_copy(ob, pss[(n, g)])
                            else:
                                nc.scalar.copy(ob, pss[(n, g)])
                            dma_eng.dma_start(
                                out=dst,
                                in_=ob.rearrange("p (r x) -> p r x", r=RG))

            if reps == 1:
                body()
            elif hoist_in:
                pre = load()
                with tc.For_i(0, reps):
                    body(pre)
            else:
                with tc.For_i(0, reps):
                    body()

    if DEDUP:
        _dedup_ldweights(nc)
    nc.finalize()
    return nc


def build(reps=1, **kw):
    return _build(reps=reps, **kw)


def _prep(img: np.ndarray, filtro: np.ndarray):
    from ml_dtypes import bfloat16

    img_bf = np.ascontiguousarray(
        np.asarray(img, dtype=np.float32)).astype(bfloat16)
    filtro = np.asarray(filtro, dtype=np.float32)
    # w[ci, ((cb*9 + ki*3+kj)*128 + co128] = filtro[cb*128+co128, ci, ki, kj]
    wt = np.transpose(filtro, (1, 2, 3, 0)).reshape(C_IN, K, K, 2, 128)
    wt = np.ascontiguousarray(np.transpose(wt, (0, 3, 1, 2, 4))).reshape(
        C_IN, 2 * K * K * 128).astype(bfloat16)
    return img_bf, wt


def kernel(img: np.ndarray, filtro: np.ndarray) -> np.ndarray:
    from concourse.bass_utils import run_bass_kernel_spmd

    img_bf, wt = _prep(img, filtro)

    if "nc" not in _CACHE:
        _CACHE["nc"] = _build()
    nc = _CACHE["nc"]

    in_maps = [
        {"img": np.ascontiguousarray(img_bf[c * PER:(c + 1) * PER]),
         "w": wt}
        for c in range(N_CORES)
    ]
    res = run_bass_kernel_spmd(nc, in_maps, list(range(N_CORES)))
    return np.concatenate(
        [res.results[c]["out"] for c in range(N_CORES)], axis=0)



# revision 2
# speedup vs baseline: 1.0824x; 1.0824x over previous
"""3x3 VALID conv (NCHW) on 8 Trainium2 NeuronCores — mixed bf16/fp8e4.

Contract: kernel(img, filtro) takes the FULL inputs
  img    [32, 128, 56, 56] f32
  filtro [256, 128, 3, 3]  f32
and returns the FULL output [32, 256, 54, 54] f32.

Strategy (per core, batch shard of 4 images): the 9-tap x 128-channel
contraction for each 128-wide co block accumulates in one PSUM bank as
  - 7 taps as bf16 504-col matmuls (weights pre-scaled * 2^12), and
  - 2 taps (FP8_PAIR) fused into ONE fp8e4 DoubleRow matmul: K=256 via
    2 k-tiles = the two taps' e4m3 img slices (constant flat-offset
    delta between the taps), weights e4m3(w*2^12).
The moving operand streams at 2 bytes/cycle/partition, so the fp8
DoubleRow pair runs in half the cycles of two bf16 taps: PE columns
drop ~11% vs all-bf16. Measured end-to-end rel err 1.84e-2 (gate
2e-2); all-bf16 is 2.2e-3. PSUM holds out*2^12; the DVE drain
multiplies by 2^-12.

Groups are 9 output rows processed as flat 504-col matmuls (rows kept
56 wide; the 2 wraparound cols/row are stored to DRAM and cropped on
host during the unshard). Waves of 4 groups alternate the 8 PSUM
banks, so wave i+2's matmuls only reuse a bank a full wave after its
drain started. A post-Tile IR pass (_dedup_ldweights) drops Ldweights
that reload the stationary already in the PE array (step-outer waves
reuse each load 4x). Out-DMA rides the ACT ring (nc.scalar.dma_start)
so stores never block the SP ring that prefetches the next
iteration's images.

Measured (For_i slope, 8 cores): ~105-110us/rep steady state vs a
~101us pure-matmul floor; the all-bf16 predecessor was ~117us
(graded 121954ns) with a ~112us floor.
"""
from contextlib import ExitStack

import numpy as np

BATCH, C_IN, C_OUT, H, K = 32, 128, 256, 56, 3
OH = H - K + 1   # 54
N_CORES = 8
PER = BATCH // N_CORES   # 4
RG = 9                   # output rows per group
NG = OH // RG            # 6
PLANE = H * H            # 3136 elems per image plane
IMG_LEN = PER * PLANE    # 12544
FP8_PAIR = (2, 4)        # taps (ki,kj)=(0,2),(1,1): flat offsets 2, 57
TAP_OFF = [0, 1, 2, 56, 57, 58, 112, 113, 114]
BF_TAPS = [t for t in range(9) if t not in FP8_PAIR]

_CACHE = {}
DEDUP = True


def _dedup_ldweights(nc):
    """Drop Ldweights that reload the stationary already in the PE array
    (legalization emits one per matmul; step-outer waves reuse each load
    wave_sz times). Waits/updates of removed loads move to the next PE
    instruction."""
    removed = 0
    for blk in nc.m.functions[0].blocks:
        insts = list(blk.instructions)
        last_w = None
        drop, pending = [], []
        for i, inst in enumerate(insts):
            if str(getattr(inst, "engine", "")) != "EngineType.PE":
                continue
            if pending and inst.opcode in ("Matmult", "Ldweights"):
                si = inst.sync_info
                waits = list(si.on_wait) if si else []
                ups = list(si.on_update) if si else []
                from concourse import mybir as _mb
                for psi in pending:
                    waits += list(psi.on_wait)
                    ups += list(psi.on_update)
                inst.sync_info = _mb.SyncInfo(on_wait=waits, on_update=ups)
                pending = []
            if inst.opcode == "Ldweights":
                sig = str(inst.ins[0])
                if sig == last_w:
                    si = inst.sync_info
                    if si and (len(si.on_wait) or len(si.on_update)):
                        pending.append(si)
                    drop.append(i)
                else:
                    last_w = sig
            elif inst.opcode == "Matmult":
                if inst.ldweights is not False:
                    last_w = str(inst.ins[1])
            else:
                last_w = None
        assert not pending
        for i in reversed(drop):
            del blk.instructions[i]
        removed += len(drop)
    return removed


def _build(reps=1, drop_out=False, drop_mm=False, fp8_taps=2,
           wave_sz=4, in_bufs=2, drain_engine="vector", out_ring="scalar",
           bigdma=False, hoist_in=False):
    import concourse.tile as tile
    from concourse import bacc, mybir
    from concourse.ap import AP

    F8 = mybir.dt.float8e4
    BF = mybir.dt.bfloat16
    F32 = mybir.dt.float32
    DRMODE = mybir.MatmulPerfMode.DoubleRow
    assert fp8_taps in (0, 2)
    n_bf = 9 if fp8_taps == 0 else 7
    bf_taps = list(range(9)) if fp8_taps == 0 else BF_TAPS
    pair_base = TAP_OFF[FP8_PAIR[0]]
    pair_delta = TAP_OFF[FP8_PAIR[1]] - TAP_OFF[FP8_PAIR[0]]

    nc = bacc.Bacc(None, target_bir_lowering=False)
    imgb = nc.declare_dram_parameter("imgb", [C_IN, IMG_LEN], BF,
                                     isOutput=False)
    wb = nc.declare_dram_parameter("wb", [C_IN, 2 * n_bf * 128], BF,
                                   isOutput=False)
    if fp8_taps:
        imgq = nc.declare_dram_parameter("imgq", [C_IN, IMG_LEN], F8,
                                         isOutput=False)
        wq = nc.declare_dram_parameter("wq", [C_IN, 2 * 2 * 128], F8,
                                       isOutput=False)
    # rows stored 56 wide; host crops the 2 garbage cols per row
    out = nc.declare_dram_parameter("out", [PER, C_OUT, OH, H], F32,
                                    isOutput=True)

    with tile.TileContext(nc) as tc:
        with ExitStack() as ctx:
            wpool = ctx.enter_context(
                tc.tile_pool(name="wpool", bufs=in_bufs))
            imgpool = ctx.enter_context(
                tc.tile_pool(name="imgpool", bufs=in_bufs))
            psum_pool = ctx.enter_context(
                tc.tile_pool(name="psum", bufs=8, space="PSUM"))
            outp = ctx.enter_context(tc.tile_pool(
                name="outp", bufs=4 if bigdma else 8))

            def load():
                wq_sb = None
                iq_sb = None
                if fp8_taps:
                    wq_sb = wpool.tile([C_IN, 2 * 2 * 128], F8)
                    nc.sync.dma_start(out=wq_sb, in_=wq[:, :])
                wb_sb = wpool.tile([C_IN, 2 * n_bf * 128], BF)
                nc.sync.dma_start(out=wb_sb, in_=wb[:, :])
                if fp8_taps:
                    iq_sb = imgpool.tile([C_IN, IMG_LEN + 64], F8)
                ib_sb = imgpool.tile([C_IN, IMG_LEN + 64], BF)
                for n in range(PER):
                    if fp8_taps:
                        nc.sync.dma_start(
                            out=iq_sb[:, n * PLANE:(n + 1) * PLANE],
                            in_=imgq[:, n * PLANE:(n + 1) * PLANE])
                    nc.sync.dma_start(
                        out=ib_sb[:, n * PLANE:(n + 1) * PLANE],
                        in_=imgb[:, n * PLANE:(n + 1) * PLANE])
                return wq_sb, wb_sb, iq_sb, ib_sb

            def body(preloaded=None):
                wq_sb, wb_sb, iq_sb, ib_sb = (preloaded if preloaded
                                              else load())
                groups = [(n, g) for n in range(PER) for g in range(NG)]
                for cb in range(2):
                    for w0 in range(0, len(groups), wave_sz):
                        wave = groups[w0:w0 + wave_sz]
                        pss = {ng: psum_pool.tile([128, RG * H], F32,
                                                  name=f"ps{ng}", tag="ps")
                               for ng in wave}
                        if not drop_mm:
                            nstep = n_bf + (1 if fp8_taps else 0)
                            si = 0
                            if fp8_taps:
                                lhsT = wq_sb[
                                    :, cb * 256:(cb + 1) * 256].rearrange(
                                    "p (two co) -> p two co", two=2)
                                for (n, g) in wave:
                                    o = (iq_sb.offset + n * PLANE
                                         + g * RG * H + pair_base)
                                    rhs = AP(iq_sb.tensor, o,
                                             [[iq_sb.ap[0][0], C_IN],
                                              [pair_delta, 2], [1, RG * H]])
                                    nc.tensor.matmul(
                                        pss[(n, g)], lhsT, rhs,
                                        start=True, stop=False,
                                        perf_mode=DRMODE)
                                si = 1
                            for i, t in enumerate(bf_taps):
                                col = (cb * n_bf + i) * 128
                                lhsT = wb_sb[:, col:col + 128]
                                for (n, g) in wave:
                                    o = (ib_sb.offset + n * PLANE
                                         + g * RG * H + TAP_OFF[t])
                                    rhs = AP(ib_sb.tensor, o,
                                             [[ib_sb.ap[0][0], C_IN],
                                              [1, RG * H]])
                                    nc.tensor.matmul(
                                        pss[(n, g)], lhsT, rhs,
                                        start=(si == 0),
                                        stop=(si == nstep - 1))
                                si += 1
                        if drop_out or drop_mm:
                            continue
                        dma_eng = (nc.scalar if out_ring == "scalar"
                                   else nc.sync)
                        if bigdma:
                            runs = []
                            for (n, g) in wave:
                                if runs and runs[-1][0] == n:
                                    runs[-1][1].append(g)
                                else:
                                    runs.append((n, [g]))
                            for n, gs in runs:
                                ob = outp.tile([128, len(gs), RG, H],
                                               F32, name="ob", tag="ob")
                                for i, g in enumerate(gs):
                                    src = pss[(n, g)].rearrange(
                                        "p (r x) -> p r x", r=RG)
                                    nc.vector.tensor_scalar_mul(
                                        ob[:, i], src, 2.0 ** -12)
                                nc.scalar.dma_start(
                                    out=out[n, cb * 128:(cb + 1) * 128,
                                            gs[0] * RG:
                                            (gs[0] + len(gs)) * RG],
                                    in_=ob.rearrange(
                                        "p a r x -> p (a r) x"))
                            continue
                        for j, (n, g) in enumerate(wave):
                            ob = outp.tile([128, RG, H], F32,
                                           name="ob", tag="ob")
                            src = pss[(n, g)].rearrange(
                                "p (r x) -> p r x", r=RG)
                            eng = drain_engine
                            if eng == "alt":
                                eng = "vector" if j % 2 == 0 else "scalar"
                            if eng == "vector":
                                nc.vector.tensor_scalar_mul(
                                    ob, src, 2.0 ** -12)
                            else:
                                nc.scalar.mul(ob, src, 2.0 ** -12)
                            dma_eng.dma_start(
                                out=out[n, cb * 128:(cb + 1) * 128,
                                        g * RG:(g + 1) * RG],
                                in_=ob)

            if reps == 1:
                body()
            elif hoist_in:
                pre = load()
                with tc.For_i(0, reps):
                    body(pre)
            else:
                with tc.For_i(0, reps):
                    body()

    if DEDUP:
        _dedup_ldweights(nc)
    nc.finalize()
    return nc


def build(reps=1, **kw):
    return _build(reps=reps, **kw)


def _prep(img: np.ndarray, filtro: np.ndarray, fp8_taps=2):
    from ml_dtypes import float8_e4m3 as e4m3, bfloat16

    x = np.asarray(img, dtype=np.float32)          # [32, 128, 56, 56]
    w = np.asarray(filtro, dtype=np.float32)       # [256, 128, 3, 3]

    imgb = np.transpose(x.astype(bfloat16),
                        (1, 0, 2, 3)).reshape(C_IN, BATCH * PLANE)
    imgq = np.transpose(x.astype(e4m3),
                        (1, 0, 2, 3)).reshape(C_IN, BATCH * PLANE)

    bf_taps = list(range(9)) if fp8_taps == 0 else BF_TAPS
    # A[cb, co, ci, t]
    wbs = (w * 4096.0).astype(bfloat16).reshape(2, 128, C_IN, 9)
    wqs = (w * 4096.0).astype(e4m3).reshape(2, 128, C_IN, 9)
    wb = np.zeros((C_IN, 2, len(bf_taps), 128), bfloat16)
    for i, t in enumerate(bf_taps):
        wb[:, :, i, :] = np.transpose(wbs[:, :, :, t], (2, 0, 1))
    wb = wb.reshape(C_IN, 2 * len(bf_taps) * 128)
    wq = np.zeros((C_IN, 2, 2, 128), e4m3)
    for k in range(2):
        wq[:, :, k, :] = np.transpose(wqs[:, :, :, FP8_PAIR[k]], (2, 0, 1))
    wq = wq.reshape(C_IN, 2 * 2 * 128)
    return imgb, imgq, np.ascontiguousarray(wb), np.ascontiguousarray(wq)


def _in_maps(imgb, imgq, wb, wq, fp8_taps=2):
    maps = []
    for c in range(N_CORES):
        sl = slice(c * PER * PLANE, (c + 1) * PER * PLANE)
        m = {"imgb": np.ascontiguousarray(imgb[:, sl]), "wb": wb}
        if fp8_taps:
            m["imgq"] = np.ascontiguousarray(imgq[:, sl])
            m["wq"] = wq
        maps.append(m)
    return maps


def kernel(img: np.ndarray, filtro: np.ndarray) -> np.ndarray:
    from concourse.bass_utils import run_bass_kernel_spmd

    imgb, imgq, wb, wq = _prep(img, filtro)

    if "nc" not in _CACHE:
        _CACHE["nc"] = _build()
    nc = _CACHE["nc"]

    res = run_bass_kernel_spmd(nc, _in_maps(imgb, imgq, wb, wq),
                               list(range(N_CORES)))
    full = np.concatenate(
        [res.results[c]["out"] for c in range(N_CORES)], axis=0)
    # each stored row is 56 wide; cols 54,55 are conv wraparound garbage,
    # dropped here during the unshard.
    return np.ascontiguousarray(full[:, :, :, :OH])
